# revision 32
# baseline (speedup 1.0000x reference)
"""Distributed Trainium2 kernel for AdaptiveAttentionBlock.

Reference computation (B=2, S=2048, H=1024, NH=16, DH=64):
    q/k/v = hidden @ W{q,k,v}.T + b      (per-head split)
    scores = q k^T / 8 + (1-mask)*-1e4
    probs  = softmax(scores) * attention_weights[key]
    ctx    = probs @ v ; out = ctx @ Wo.T + bo
    normed = LayerNorm(out + hidden) * gamma + beta
    returns (normed, probs)

Sharding: 8 cores = 2 batches x 4 head-groups. Core c=4b+g computes heads
4g..4g+3 of batch b and a partial (head-group slice of the contraction) of
out = ctx @ Wo.T for all tokens; a ReduceScatter over the 4-core batch group
sums the partials and hands core g the token block 512g..512(g+1), on which
it applies residual + LayerNorm.

Tricks:
  - ln(attention_weights) + mask penalty folded into an extra contraction row
    of the scores matmul, so probs = exp(scores') * invsum with one exp pass.
  - row-sum of the unweighted exp obtained via an extra 1/aw column in v.
  - float32r matmuls (full-rate fp32), bf16 for e^T / v / Wo operands.
"""

import numpy as np

B, S, H = 2, 2048, 1024
NH, DH = 16, 64
G = 4              # head-groups (tensor-parallel dim)
HPG = NH // G      # 4 heads per core
DG = HPG * DH      # 256 channels per group
TB = S // G        # 512 output tokens per core
NQB = S // 128     # 16 query blocks
NSC = S // 128     # 16 key/seq chunks
EPS = 1e-12

_CACHE = {}


def _build_nc(single_core=False):
    from concourse import bacc, tile, mybir

    FP = mybir.dt.float32
    BF = mybir.dt.bfloat16
    FR = mybir.dt.float32r
    AF = mybir.ActivationFunctionType
    ALU = mybir.AluOpType

    nc = bacc.Bacc("TRN2", target_bir_lowering=False, debug=False,
                   num_devices=1 if single_core else 8)

    # ---- kernel I/O (per-core shards, host pre-laid-out for contiguous DMA)
    xt_d = nc.dram_tensor("xt", [128, 8, S], BF, kind="ExternalInput")
    xres_d = nc.dram_tensor("xres", [128, 4, H], FP, kind="ExternalInput")
    wq_d = nc.dram_tensor("wq", [128, 8, DG], BF, kind="ExternalInput")
    wk_d = nc.dram_tensor("wk", [128, 8, DG], BF, kind="ExternalInput")
    wv_d = nc.dram_tensor("wv", [128, 8, DG], BF, kind="ExternalInput")
    wo_d = nc.dram_tensor("wo", [128, 2, H], BF, kind="ExternalInput")
    bq_d = nc.dram_tensor("bq", [1, DG], BF, kind="ExternalInput")
    bk_d = nc.dram_tensor("bk", [1, DG], BF, kind="ExternalInput")
    bv_d = nc.dram_tensor("bv", [1, DG], BF, kind="ExternalInput")
    lnaw_d = nc.dram_tensor("lnaw", [2, S], BF, kind="ExternalInput")
    invaw_d = nc.dram_tensor("invaw", [128, NSC], FP, kind="ExternalInput")
    ident_d = nc.dram_tensor("ident", [128, 128], FP, kind="ExternalInput")
    gamma_d = nc.dram_tensor("gamma", [128, H], FP, kind="ExternalInput")
    beta_d = nc.dram_tensor("beta", [128, H], FP, kind="ExternalInput")

    probs_d = nc.dram_tensor("probs", [HPG, S, S], BF, kind="ExternalOutput")
    norm_d = nc.dram_tensor("norm", [TB, H], FP, kind="ExternalOutput")

    with tile.TileContext(nc) as tc, tc.tile_pool(name="persist", bufs=1) as pers:
        # ---- resident SBUF tensors (whole-kernel lifetime)
        bq_sb = pers.tile([1, DG], BF, tag="bq", name="bq_sb")
        bk_sb = pers.tile([1, DG], BF, tag="bk", name="bk_sb")
        bv_sb = pers.tile([1, DG], BF, tag="bv", name="bv_sb")
        lnaw_sb = pers.tile([2, S], BF, tag="lnaw", name="lnaw_sb")
        invaw_sb = pers.tile([128, NSC], FP, tag="invaw", name="invaw_sb")
        ident_sb = pers.tile([128, 128], FP, tag="ident", name="ident_sb")
        identb_sb = pers.tile([128, 128], BF, tag="identb", name="identb_sb")
        ones_sb = pers.tile([2, S], BF, tag="ones", name="ones_sb")
        eps_sb = pers.tile([128, 1], FP, tag="eps", name="eps_sb")
        wo_sb = pers.tile([128, 2, H], BF, tag="wo", name="wo_sb")

        # [64*(h%2)+d, h//2, s] layout for q^T, k^T and ctx^T
        q_sb = pers.tile([128, 2, S], BF, tag="q", name="q_sb")
        k_sb = pers.tile([128, 2, S], BF, tag="k", name="k_sb")
        v_sb = pers.tile([128, NSC, HPG * (DH + 1)], BF, tag="v", name="v_sb")
        ctxT_sb = pers.tile([128, 2, S], BF, tag="ctxT", name="ctxT_sb")

        nc.sync.dma_start(out=bq_sb[:], in_=bq_d.ap())
        nc.sync.dma_start(out=bk_sb[:], in_=bk_d.ap())
        nc.sync.dma_start(out=bv_sb[:], in_=bv_d.ap())
        nc.sync.dma_start(out=lnaw_sb[:], in_=lnaw_d.ap())
        nc.sync.dma_start(out=invaw_sb[:], in_=invaw_d.ap())
        nc.vector.memset(ones_sb[:], 1.0)
        nc.vector.memset(eps_sb[:], EPS)
        warm_sb = pers.tile([1, 1], FP, tag="warm", name="warm_sb")
        nc.scalar.activation(warm_sb[:], eps_sb[0:1, 0:1], AF.Exp)

        with tc.tile_pool(name="ps_small", bufs=2, space="PSUM") as ps_small:
            # ================= QKV projections =================
            # (tiles live alongside the attention pools; pair-1 q/k emitted
            # inside the block pipeline so attention starts on pair 0 early)
            if True:
                xt_sb = pers.tile([128, 8, S], BF, tag="xt", name="xt_sb")
                wq_sb = pers.tile([128, 8, DG], BF, tag="wq", name="wq_sb")
                wk_sb = pers.tile([128, 8, DG], BF, tag="wk", name="wk_sb")
                wv_sb = pers.tile([128, 8, DG], BF, tag="wv", name="wv_sb")
                nc.sync.dma_start(out=wv_sb[:], in_=wv_d.ap())
                nc.sync.dma_start(out=wq_sb[:], in_=wq_d.ap())
                for c in range(8):
                    nc.sync.dma_start(out=xt_sb[:, c, :], in_=xt_d.ap()[:, c, :])
                nc.sync.dma_start(out=wk_sb[:], in_=wk_d.ap())
                nc.scalar.dma_start(out=ident_sb[:], in_=ident_d.ap())
                nc.scalar.dma_start(out=wo_sb[:], in_=wo_d.ap())
                nc.vector.tensor_copy(identb_sb[:], ident_sb[:])

                def emit_v():
                    # v: [s, d] natural layout + 1/aw column per head (bf16)
                    for sc in range(NSC):
                        ps = ps_small.tile([128, DG], FP, tag="small",
                                           name=f"ps_v{sc}")
                        for c in range(8):
                            nc.tensor.matmul(
                                ps[:],
                                lhsT=xt_sb[:, c, 128 * sc:128 * sc + 128],
                                rhs=wv_sb[:, c, :],
                                start=(c == 0), stop=False,
                            )
                        nc.tensor.matmul(
                            ps[:],
                            lhsT=ones_sb[0:1, 0:128],
                            rhs=bv_sb[0:1, :],
                            start=False, stop=True,
                        )
                        for h in range(HPG):
                            nc.vector.tensor_copy(
                                v_sb[:, sc, 65 * h:65 * h + 64],
                                ps[:, 64 * h:64 * h + 64])
                            nc.vector.tensor_copy(
                                v_sb[:, sc, 65 * h + 64:65 * h + 65],
                                invaw_sb[:, sc:sc + 1])

                def emit_qk(pr, which):
                    # qT/kT [d, s], 2 head-pairs stacked per partition dim
                    w_sb, b_sb, dst = ((wq_sb, bq_sb, q_sb) if which == "q"
                                       else (wk_sb, bk_sb, k_sb))
                    for sj in range(4):            # 512-wide s chunk
                        ps = ps_small.tile([128, 512], FP, tag="small",
                                           name=f"ps_{which}{pr}{sj}")
                        for c in range(8):
                            nc.tensor.matmul(
                                ps[:],
                                lhsT=w_sb[:, c, 128 * pr:128 * pr + 128],
                                rhs=xt_sb[:, c, 512 * sj:512 * sj + 512],
                                start=(c == 0), stop=False,
                            )
                        # bias row: out[d, s] += b[d] * 1[s]
                        nc.tensor.matmul(
                            ps[:],
                            lhsT=b_sb[0:1, 128 * pr:128 * pr + 128],
                            rhs=ones_sb[0:1, 512 * sj:512 * sj + 512],
                            start=False, stop=True,
                        )
                        nc.vector.tensor_copy(
                            dst[:, pr, 512 * sj:512 * sj + 512], ps[:])

                emit_v()
                emit_qk(0, "q")
                emit_qk(0, "k")
                emit_qk(1, "q")
                emit_qk(1, "k")

            # ================= attention + partial Wo =================
            with (
                tc.tile_pool(name="ps_big", bufs=2, space="PSUM") as ps_big,
                tc.tile_pool(name="ps_tr", bufs=2, space="PSUM") as ps_tr,
                tc.tile_pool(name="sb_e", bufs=3) as sb_e,
                tc.tile_pool(name="sb_eT", bufs=2) as sb_eT,
                tc.tile_pool(name="sb_probs", bufs=3) as sb_probs,
                tc.tile_pool(name="sb_small", bufs=4) as sb_small,
                tc.tile_pool(name="dram", bufs=1, space="DRAM") as dram_pool,
                tc.tile_pool(name="epi", bufs=1) as epi,
            ):
                # four token-quarter partial buffers: quarter i holds tokens
                # 512g + 128i .. +128 for every group-rank g (so that the
                # ReduceScatter of quarter i hands core g exactly its chunk)
                partials = [dram_pool.tile([TB, H], BF, name=f"partial{i}")
                            for i in range(4)]
                rss = [dram_pool.tile([128, H], BF, name=f"rs{i}")
                       for i in range(4)]

                xres_sb = epi.tile([128, 4, H], FP, tag="xres", name="xres_sb")
                gamma_sb = epi.tile([128, H], FP, tag="gam", name="gamma_sb")
                beta_sb = epi.tile([128, H], FP, tag="bet", name="beta_sb")
                nc.sync.dma_start(out=xres_sb[:], in_=xres_d.ap())
                nc.sync.dma_start(out=gamma_sb[:], in_=gamma_d.ap())
                nc.sync.dma_start(out=beta_sb[:], in_=beta_d.ap())

                def emit_scores(h, qb):
                    """scores matmul + exp; returns the e' tile (bf16)."""
                    hp, h2 = h // 2, h % 2
                    qT = q_sb[64 * h2:64 * h2 + 64, hp, 128 * qb:128 * qb + 128]
                    e_sb = sb_e.tile([128, S], BF, tag="e", name=f"e{h}_{qb}")
                    for half in range(2):
                        scores = ps_big.tile([128, 1024], FP, tag="scores",
                                             name=f"sc{h}_{qb}_{half}")
                        for sjh in range(2):
                            sj = 2 * half + sjh
                            nc.tensor.matmul(
                                scores[:, 512 * sjh:512 * sjh + 512],
                                lhsT=qT,
                                rhs=k_sb[64 * h2:64 * h2 + 64, hp,
                                         512 * sj:512 * sj + 512],
                                start=True, stop=False,
                            )
                            nc.tensor.matmul(
                                scores[:, 512 * sjh:512 * sjh + 512],
                                lhsT=ones_sb[0:2, 0:128],
                                rhs=lnaw_sb[0:2, 512 * sj:512 * sj + 512],
                                start=False, stop=True,
                            )
                        # e' = exp(scores + lnaw)  (aw-weighted, unnormalized)
                        nc.scalar.activation(
                            e_sb[:, 1024 * half:1024 * half + 1024], scores[:],
                            AF.Exp)
                    return e_sb

                def emit_transposes(h, qb, e_sb):
                    """transpose e' into bf16 [s, q] tiles, 2 half-rounds."""
                    eT = sb_eT.tile([128, S], BF, tag="eT", name=f"eT{h}_{qb}")
                    for half in range(2):
                        tr = ps_tr.tile([128, 1024], BF, tag="tr",
                                        name=f"tr{h}_{qb}_{half}")
                        for j in range(8):
                            i = 8 * half + j
                            nc.tensor.transpose(
                                tr[:, 128 * j:128 * j + 128],
                                e_sb[:, 128 * i:128 * i + 128],
                                identb_sb[:],
                            )
                        nc.vector.tensor_copy(
                            eT[:, 1024 * half:1024 * half + 1024], tr[:])
                    return eT

                def emit_rest(h, qb, e_sb, eT):
                    """ctx matmuls, probs scale + DMA, scaled ctx^T."""
                    hp, h2 = h // 2, h % 2
                    # ctx (+rowsum col) = sum_i eT_i^T @ [v_i | 1/aw_i]
                    ctx_ps = ps_small.tile([128, DH + 1], FP, tag="small",
                                           name=f"cx{h}_{qb}")
                    for i in range(NSC):
                        nc.tensor.matmul(
                            ctx_ps[:],
                            lhsT=eT[:, 128 * i:128 * i + 128],
                            rhs=v_sb[:, i, 65 * h:65 * h + 65],
                            start=(i == 0), stop=(i == NSC - 1),
                        )
                    invsum = sb_small.tile([128, 1], FP, tag="inv",
                                           name=f"iv{h}_{qb}")
                    nc.vector.reciprocal(invsum[:], ctx_ps[:, DH:DH + 1])

                    # probs = e' * invsum  -> DMA out
                    probs_sb = sb_probs.tile([128, S], BF, tag="probs",
                                             name=f"pb{h}_{qb}")
                    nc.vector.tensor_scalar_mul(probs_sb[:], e_sb[:], invsum[:])
                    nc.sync.dma_start(
                        out=probs_d.ap()[h, 128 * qb:128 * qb + 128, :],
                        in_=probs_sb[:])

                    # ctx scaled + transposed into persistent ctx^T
                    ctx_sb = sb_small.tile([128, DH], BF, tag="ctx",
                                           name=f"cs{h}_{qb}")
                    nc.vector.tensor_scalar_mul(ctx_sb[:], ctx_ps[:, 0:DH],
                                                invsum[:])
                    ctxT_ps = ps_small.tile([DH, 128], BF, tag="small",
                                            name=f"ct{h}_{qb}")
                    nc.tensor.transpose(ctxT_ps[:], ctx_sb[:], identb_sb[:])
                    nc.vector.tensor_copy(
                        ctxT_sb[64 * h2:64 * h2 + 64, hp,
                                128 * qb:128 * qb + 128],
                        ctxT_ps[:])

                def emit_wo(qb):
                    # partial out = ctx^T(qb)^T @ Wo^T  for token block qb,
                    # written to quarter buffer qb%4 at group-rank-major rows
                    psb = sb_probs.tile([128, H], BF, tag="partial",
                                        name=f"po{qb}")
                    for nk in range(2):
                        ps = ps_small.tile([128, 512], FP, tag="small",
                                           name=f"wo{qb}{nk}")
                        for cc in range(2):
                            nc.tensor.matmul(
                                ps[:],
                                lhsT=ctxT_sb[:, cc, 128 * qb:128 * qb + 128],
                                rhs=wo_sb[:, cc, 512 * nk:512 * nk + 512],
                                start=(cc == 0), stop=(cc == 1),
                            )
                        nc.scalar.copy(psb[:, 512 * nk:512 * nk + 512], ps[:])
                    g, i = divmod(qb, 4)
                    nc.sync.dma_start(
                        out=partials[i][128 * g:128 * g + 128, :], in_=psb[:])

                rs_sbs = {}

                def emit_rs(i):
                    # ReduceScatter quarter i + readback, all on gpsimd so no
                    # compute-engine FIFO ever waits on the collective chain
                    if single_core:
                        nc.gpsimd.dma_start(out=rss[i][:, :],
                                            in_=partials[i][0:128, :])
                    else:
                        nc.gpsimd.collective_compute(
                            "ReduceScatter",
                            ALU.add,
                            replica_groups=[[0, 1, 2, 3], [4, 5, 6, 7]],
                            ins=[partials[i].opt()],
                            outs=[rss[i].opt()],
                        )
                    rs_sb = sb_probs.tile([128, H], BF, tag="partial",
                                          name=f"rsb{i}")
                    nc.gpsimd.dma_start(out=rs_sb[:], in_=rss[i][:, :])
                    rs_sbs[i] = rs_sb

                def emit_ln(i):
                    rs_sb = rs_sbs[i]
                    res = sb_e.tile([128, H], FP, tag="res", name=f"res{i}")
                    nc.vector.tensor_tensor(
                        res[:], rs_sb[:], xres_sb[:, i, :], ALU.add)
                    stats = sb_small.tile([128, 2, 6], FP, tag="st",
                                          name=f"st{i}")
                    mv = sb_small.tile([128, 2], FP, tag="mv", name=f"mv{i}")
                    for a in range(2):
                        nc.vector.bn_stats(
                            stats[:, a, :], res[:, 512 * a:512 * a + 512])
                    nc.vector.bn_aggr(mv[:], stats[:])
                    lnv = sb_small.tile([128, 1], FP, tag="inv", name=f"lv{i}")
                    nc.scalar.activation(lnv[:], mv[:, 1:2], AF.Ln, bias=eps_sb[:])
                    rstd = sb_small.tile([128, 1], FP, tag="inv", name=f"rs{i}")
                    nc.scalar.activation(rstd[:], lnv[:], AF.Exp, scale=-0.5)
                    nrm = sb_e.tile([128, H], FP, tag="nrm", name=f"nm{i}")
                    nc.vector.tensor_scalar(
                        nrm[:], res[:], mv[:, 0:1], rstd[:],
                        ALU.subtract, ALU.mult)
                    nc.vector.tensor_tensor(nrm[:], nrm[:], gamma_sb[:], ALU.mult)
                    nc.vector.tensor_tensor(nrm[:], nrm[:], beta_sb[:], ALU.add)
                    nc.sync.dma_start(
                        out=norm_d.ap()[128 * i:128 * i + 128, :], in_=nrm[:])

                # software pipeline: while ACT runs exp(b), PE runs the
                # transposes/ctx of block b-1 instead of stalling. qb order is
                # residue-major so each quarter's ReduceScatter launches early.
                qb_order = sorted(range(NQB), key=lambda q: (q % 4, q // 4))
                # pair-0 heads of the first 3 q-blocks run first so attention
                # starts while pair-1 q/k (emitted mid-pipeline) compute
                blocks = [(h, qb) for qb in qb_order for h in range(HPG)]
                done_in_residue = [0, 0, 0, 0]
                rest_done = {qb: 0 for qb in range(NQB)}

                def after_wo(qb):
                    i = qb % 4
                    done_in_residue[i] += 1
                    if done_in_residue[i] == 4:
                        emit_rs(i)
                        if i >= 1:
                            emit_ln(i - 1)

                def finish(entry):
                    emit_rest(*entry)
                    qb = entry[1]
                    rest_done[qb] += 1
                    if rest_done[qb] == HPG:
                        emit_wo(qb)
                        after_wo(qb)

                p1 = p2 = None     # p1: needs transposes; p2: needs rest
                for (h, qb) in blocks:
                    e_sb = emit_scores(h, qb)
                    if p1 is not None:
                        eT = emit_transposes(*p1)
                        if p2 is not None:
                            finish(p2)
                        p2 = (*p1, eT)
                    p1 = (h, qb, e_sb)
                eT = emit_transposes(*p1)
                finish(p2)
                finish((*p1, eT))
                emit_ln(3)

    nc.compile()
    return nc


def _host_inputs(hidden_states, attention_mask, attention_weights,
                 Wq, bq, Wk, bk, Wv, bv, Wo, bo, gamma, beta):
    import ml_dtypes
    f32 = np.float32
    bf16 = ml_dtypes.bfloat16
    hs = np.asarray(hidden_states, f32)
    mask = np.asarray(attention_mask)
    aw = np.asarray(attention_weights, f32)
    Wq, bq = np.asarray(Wq, f32), np.asarray(bq, f32)
    Wk, bk = np.asarray(Wk, f32), np.asarray(bk, f32)
    Wv, bv = np.asarray(Wv, f32), np.asarray(bv, f32)
    Wo, bo = np.asarray(Wo, f32), np.asarray(bo, f32)
    gamma, beta = np.asarray(gamma, f32), np.asarray(beta, f32)

    ident = np.eye(128, dtype=f32)
    scale = f32(1.0 / np.sqrt(DH))

    def chunk_pf(a, parts=128):
        # [C*parts, F...] -> [parts, C, F...] contiguous
        C = a.shape[0] // parts
        return np.ascontiguousarray(
            a.reshape(C, parts, *a.shape[1:]).transpose(1, 0, *range(2, a.ndim + 1)))

    in_maps = []
    for core in range(8):
        b, g = divmod(core, G)
        aw_b = np.maximum(aw[b], 1e-20)
        lnaw = np.log(aw_b) + (1.0 - mask[b].astype(f32)) * f32(-10000.0)
        sl = slice(DG * g, DG * g + DG)
        lnaw_hi = lnaw.astype(bf16)
        lnaw_lo = (lnaw - lnaw_hi.astype(f32)).astype(bf16)
        in_maps.append({
            "xt": chunk_pf(np.ascontiguousarray(hs[b].T)).astype(bf16),
            "xres": chunk_pf(hs[b, TB * g:TB * g + TB] + bo[None, :]),
            "wq": chunk_pf(np.ascontiguousarray(Wq[sl].T) * scale).astype(bf16),
            "wk": chunk_pf(np.ascontiguousarray(Wk[sl].T)).astype(bf16),
            "wv": chunk_pf(np.ascontiguousarray(Wv[sl].T)).astype(bf16),
            "wo": chunk_pf(np.ascontiguousarray(Wo[:, sl].T)).astype(bf16),
            "bq": (bq[sl] * scale).reshape(1, DG).astype(bf16),
            "bk": bk[sl].reshape(1, DG).astype(bf16),
            "bv": bv[sl].reshape(1, DG).astype(bf16),
            "lnaw": np.stack([lnaw_hi, lnaw_lo]).reshape(2, S),
            "invaw": np.ascontiguousarray((1.0 / aw_b).reshape(NSC, 128).T),
            "ident": ident,
            "gamma": np.ascontiguousarray(np.broadcast_to(gamma[None, :], (128, H))),
            "beta": np.ascontiguousarray(np.broadcast_to(beta[None, :], (128, H))),
        })
    return in_maps


def kernel(**inputs):
    from concourse.bass_utils import run_bass_kernel_spmd

    if "nc" not in _CACHE:
        _CACHE["nc"] = _build_nc()
    nc = _CACHE["nc"]

    in_maps = _host_inputs(**inputs)
    res = run_bass_kernel_spmd(nc, in_maps, core_ids=list(range(8)))
    outs = res.results

    normed = np.empty((B, S, H), np.float32)
    probs = np.empty((B, NH, S, S), np.float32)
    for core in range(8):
        b, g = divmod(core, G)
        normed[b, TB * g:TB * g + TB, :] = outs[core]["norm"]
        probs[b, HPG * g:HPG * g + HPG] = outs[core]["probs"].astype(np.float32)
    return normed, probs


# revision 35
# speedup vs baseline: 2.9869x; 2.9869x over previous
"""Distributed Trainium2 kernel for AdaptiveAttentionBlock.

Reference computation (B=2, S=2048, H=1024, NH=16, DH=64):
    q/k/v = hidden @ W{q,k,v}.T + b      (per-head split)
    scores = q k^T / 8 + (1-mask)*-1e4
    probs  = softmax(scores) * attention_weights[key]
    ctx    = probs @ v ; out = ctx @ Wo.T + bo
    normed = LayerNorm(out + hidden) * gamma + beta
    returns (normed, probs)

Sharding: 8 cores = 2 batches x 4 head-groups. Core c=4b+g computes heads
4g..4g+3 of batch b and a partial (head-group slice of the contraction) of
out = ctx @ Wo.T for all tokens; a ReduceScatter over the 4-core batch group
sums the partials and hands core g the token block 512g..512(g+1), on which
it applies residual + LayerNorm.

Tricks:
  - ln(attention_weights) + mask penalty folded into an extra contraction row
    of the scores matmul, so probs = exp(scores') * invsum with one exp pass.
  - row-sum of the unweighted exp obtained via an extra 1/aw column in v.
  - all matmul operands in bf16 (full-rate PE); probs stored bf16 and
    widened to f32 on the host; fp32 PSUM accumulation throughout.
"""

import numpy as np

B, S, H = 2, 2048, 1024
NH, DH = 16, 64
G = 4              # head-groups (tensor-parallel dim)
HPG = NH // G      # 4 heads per core
DG = HPG * DH      # 256 channels per group
TB = S // G        # 512 output tokens per core
NQB = S // 128     # 16 query blocks
NSC = S // 128     # 16 key/seq chunks
EPS = 1e-12

_CACHE = {}


def _build_nc(single_core=False):
    from concourse import bacc, tile, mybir

    FP = mybir.dt.float32
    BF = mybir.dt.bfloat16
    FR = mybir.dt.float32r
    AF = mybir.ActivationFunctionType
    ALU = mybir.AluOpType

    nc = bacc.Bacc("TRN2", target_bir_lowering=False, debug=False,
                   num_devices=1 if single_core else 8)

    # ---- kernel I/O (per-core shards, host pre-laid-out for contiguous DMA)
    xt_d = nc.dram_tensor("xt", [128, 8, S], BF, kind="ExternalInput")
    xres_d = nc.dram_tensor("xres", [128, 4, H], FP, kind="ExternalInput")
    wq_d = nc.dram_tensor("wq", [128, 8, DG], BF, kind="ExternalInput")
    wk_d = nc.dram_tensor("wk", [128, 8, DG], BF, kind="ExternalInput")
    wv_d = nc.dram_tensor("wv", [128, 8, DG], BF, kind="ExternalInput")
    wo_d = nc.dram_tensor("wo", [128, 2, H], BF, kind="ExternalInput")
    bq_d = nc.dram_tensor("bq", [1, DG], BF, kind="ExternalInput")
    bk_d = nc.dram_tensor("bk", [1, DG], BF, kind="ExternalInput")
    bv_d = nc.dram_tensor("bv", [1, DG], BF, kind="ExternalInput")
    lnaw_d = nc.dram_tensor("lnaw", [2, S], BF, kind="ExternalInput")
    invaw_d = nc.dram_tensor("invaw", [128, NSC], FP, kind="ExternalInput")
    ident_d = nc.dram_tensor("ident", [128, 128], FP, kind="ExternalInput")
    gamma_d = nc.dram_tensor("gamma", [128, H], FP, kind="ExternalInput")
    beta_d = nc.dram_tensor("beta", [128, H], FP, kind="ExternalInput")

    probs_d = nc.dram_tensor("probs", [HPG, S, S], BF, kind="ExternalOutput")
    norm_d = nc.dram_tensor("norm", [TB, H], FP, kind="ExternalOutput")

    with tile.TileContext(nc) as tc, tc.tile_pool(name="persist", bufs=1) as pers:
        # ---- resident SBUF tensors (whole-kernel lifetime)
        bq_sb = pers.tile([1, DG], BF, tag="bq", name="bq_sb")
        bk_sb = pers.tile([1, DG], BF, tag="bk", name="bk_sb")
        bv_sb = pers.tile([1, DG], BF, tag="bv", name="bv_sb")
        lnaw_sb = pers.tile([2, S], BF, tag="lnaw", name="lnaw_sb")
        invaw_sb = pers.tile([128, NSC], FP, tag="invaw", name="invaw_sb")
        ident_sb = pers.tile([128, 128], FP, tag="ident", name="ident_sb")
        identb_sb = pers.tile([128, 128], BF, tag="identb", name="identb_sb")
        ones_sb = pers.tile([2, S], BF, tag="ones", name="ones_sb")
        eps_sb = pers.tile([128, 1], FP, tag="eps", name="eps_sb")
        wo_sb = pers.tile([128, 2, H], BF, tag="wo", name="wo_sb")

        # [64*(h%2)+d, h//2, s] layout for q^T, k^T and ctx^T
        q_sb = pers.tile([128, 2, S], BF, tag="q", name="q_sb")
        k_sb = pers.tile([128, 2, S], BF, tag="k", name="k_sb")
        v_sb = pers.tile([128, NSC, HPG * (DH + 1)], BF, tag="v", name="v_sb")
        ctxT_sb = pers.tile([128, 2, S], BF, tag="ctxT", name="ctxT_sb")

        nc.sync.dma_start(out=bq_sb[:], in_=bq_d.ap())
        nc.sync.dma_start(out=bk_sb[:], in_=bk_d.ap())
        nc.sync.dma_start(out=bv_sb[:], in_=bv_d.ap())
        nc.sync.dma_start(out=lnaw_sb[:], in_=lnaw_d.ap())
        nc.sync.dma_start(out=invaw_sb[:], in_=invaw_d.ap())
        nc.vector.memset(ones_sb[:], 1.0)
        nc.vector.memset(eps_sb[:], EPS)
        warm_sb = pers.tile([1, 1], FP, tag="warm", name="warm_sb")
        nc.scalar.activation(warm_sb[:], eps_sb[0:1, 0:1], AF.Exp)

        with tc.tile_pool(name="ps_small", bufs=2, space="PSUM") as ps_small:
            # ================= QKV projections =================
            # (tiles live alongside the attention pools; pair-1 q/k emitted
            # inside the block pipeline so attention starts on pair 0 early)
            if True:
                xt_sb = pers.tile([128, 8, S], BF, tag="xt", name="xt_sb")
                wq_sb = pers.tile([128, 8, DG], BF, tag="wq", name="wq_sb")
                wk_sb = pers.tile([128, 8, DG], BF, tag="wk", name="wk_sb")
                wv_sb = pers.tile([128, 8, DG], BF, tag="wv", name="wv_sb")
                nc.sync.dma_start(out=wv_sb[:], in_=wv_d.ap())
                nc.sync.dma_start(out=wq_sb[:], in_=wq_d.ap())
                for c in range(8):
                    eng = (nc.sync, nc.scalar, nc.gpsimd)[c % 3]
                    eng.dma_start(out=xt_sb[:, c, :], in_=xt_d.ap()[:, c, :])
                nc.gpsimd.dma_start(out=wk_sb[:], in_=wk_d.ap())
                nc.scalar.dma_start(out=ident_sb[:], in_=ident_d.ap())
                nc.scalar.dma_start(out=wo_sb[:], in_=wo_d.ap())
                nc.vector.tensor_copy(identb_sb[:], ident_sb[:])

                def emit_v():
                    # v: [s, d] natural layout + 1/aw column per head (bf16)
                    for sc in range(NSC):
                        ps = ps_small.tile([128, DG], FP, tag="small",
                                           name=f"ps_v{sc}")
                        for c in range(8):
                            nc.tensor.matmul(
                                ps[:],
                                lhsT=xt_sb[:, c, 128 * sc:128 * sc + 128],
                                rhs=wv_sb[:, c, :],
                                start=(c == 0), stop=False,
                            )
                        nc.tensor.matmul(
                            ps[:],
                            lhsT=ones_sb[0:1, 0:128],
                            rhs=bv_sb[0:1, :],
                            start=False, stop=True,
                        )
                        for h in range(HPG):
                            nc.vector.tensor_copy(
                                v_sb[:, sc, 65 * h:65 * h + 64],
                                ps[:, 64 * h:64 * h + 64])
                            nc.vector.tensor_copy(
                                v_sb[:, sc, 65 * h + 64:65 * h + 65],
                                invaw_sb[:, sc:sc + 1])

                def emit_qk(pr, which):
                    # qT/kT [d, s], 2 head-pairs stacked per partition dim
                    w_sb, b_sb, dst = ((wq_sb, bq_sb, q_sb) if which == "q"
                                       else (wk_sb, bk_sb, k_sb))
                    for sj in range(4):            # 512-wide s chunk
                        ps = ps_small.tile([128, 512], FP, tag="small",
                                           name=f"ps_{which}{pr}{sj}")
                        for c in range(8):
                            nc.tensor.matmul(
                                ps[:],
                                lhsT=w_sb[:, c, 128 * pr:128 * pr + 128],
                                rhs=xt_sb[:, c, 512 * sj:512 * sj + 512],
                                start=(c == 0), stop=False,
                            )
                        # bias row: out[d, s] += b[d] * 1[s]
                        nc.tensor.matmul(
                            ps[:],
                            lhsT=b_sb[0:1, 128 * pr:128 * pr + 128],
                            rhs=ones_sb[0:1, 512 * sj:512 * sj + 512],
                            start=False, stop=True,
                        )
                        nc.vector.tensor_copy(
                            dst[:, pr, 512 * sj:512 * sj + 512], ps[:])

                emit_v()
                emit_qk(0, "q")
                emit_qk(0, "k")
                emit_qk(1, "q")
                emit_qk(1, "k")

            # ================= attention + partial Wo =================
            with (
                tc.tile_pool(name="ps_big", bufs=2, space="PSUM") as ps_big,
                tc.tile_pool(name="ps_tr", bufs=2, space="PSUM") as ps_tr,
                tc.tile_pool(name="sb_e", bufs=3) as sb_e,
                tc.tile_pool(name="sb_eT", bufs=2) as sb_eT,
                tc.tile_pool(name="sb_probs", bufs=3) as sb_probs,
                tc.tile_pool(name="sb_small", bufs=4) as sb_small,
                tc.tile_pool(name="dram", bufs=1, space="DRAM") as dram_pool,
                tc.tile_pool(name="epi", bufs=1) as epi,
            ):
                # four token-quarter partial buffers: quarter i holds tokens
                # 512g + 128i .. +128 for every group-rank g (so that the
                # ReduceScatter of quarter i hands core g exactly its chunk)
                partials = [dram_pool.tile([TB, H], BF, name=f"partial{i}")
                            for i in range(4)]
                rss = [dram_pool.tile([128, H], BF, name=f"rs{i}")
                       for i in range(4)]

                xres_sb = epi.tile([128, 4, H], FP, tag="xres", name="xres_sb")
                gamma_sb = epi.tile([128, H], FP, tag="gam", name="gamma_sb")
                beta_sb = epi.tile([128, H], FP, tag="bet", name="beta_sb")
                nc.sync.dma_start(out=xres_sb[:], in_=xres_d.ap())
                nc.sync.dma_start(out=gamma_sb[:], in_=gamma_d.ap())
                nc.sync.dma_start(out=beta_sb[:], in_=beta_d.ap())

                def emit_scores(h, qb):
                    """scores matmul + exp; returns the e' tile (bf16)."""
                    hp, h2 = h // 2, h % 2
                    qT = q_sb[64 * h2:64 * h2 + 64, hp, 128 * qb:128 * qb + 128]
                    e_sb = sb_e.tile([128, S], BF, tag="e", name=f"e{h}_{qb}")
                    for half in range(2):
                        scores = ps_big.tile([128, 1024], FP, tag="scores",
                                             name=f"sc{h}_{qb}_{half}")
                        for sjh in range(2):
                            sj = 2 * half + sjh
                            nc.tensor.matmul(
                                scores[:, 512 * sjh:512 * sjh + 512],
                                lhsT=qT,
                                rhs=k_sb[64 * h2:64 * h2 + 64, hp,
                                         512 * sj:512 * sj + 512],
                                start=True, stop=False,
                            )
                            nc.tensor.matmul(
                                scores[:, 512 * sjh:512 * sjh + 512],
                                lhsT=ones_sb[0:2, 0:128],
                                rhs=lnaw_sb[0:2, 512 * sj:512 * sj + 512],
                                start=False, stop=True,
                            )
                        # e' = exp(scores + lnaw)  (aw-weighted, unnormalized)
                        nc.scalar.activation(
                            e_sb[:, 1024 * half:1024 * half + 1024], scores[:],
                            AF.Exp)
                    return e_sb

                def emit_transposes(h, qb, e_sb):
                    """transpose e' into bf16 [s, q] tiles, 2 half-rounds."""
                    eT = sb_eT.tile([128, S], BF, tag="eT", name=f"eT{h}_{qb}")
                    for half in range(2):
                        tr = ps_tr.tile([128, 1024], BF, tag="tr",
                                        name=f"tr{h}_{qb}_{half}")
                        for j in range(8):
                            i = 8 * half + j
                            nc.tensor.transpose(
                                tr[:, 128 * j:128 * j + 128],
                                e_sb[:, 128 * i:128 * i + 128],
                                identb_sb[:],
                            )
                        nc.vector.tensor_copy(
                            eT[:, 1024 * half:1024 * half + 1024], tr[:])
                    return eT

                def emit_rest(h, qb, e_sb, eT):
                    """ctx matmuls, probs scale + DMA, scaled ctx^T."""
                    hp, h2 = h // 2, h % 2
                    # ctx (+rowsum col) = sum_i eT_i^T @ [v_i | 1/aw_i]
                    ctx_ps = ps_small.tile([128, DH + 1], FP, tag="small",
                                           name=f"cx{h}_{qb}")
                    for i in range(NSC):
                        nc.tensor.matmul(
                            ctx_ps[:],
                            lhsT=eT[:, 128 * i:128 * i + 128],
                            rhs=v_sb[:, i, 65 * h:65 * h + 65],
                            start=(i == 0), stop=(i == NSC - 1),
                        )
                    invsum = sb_small.tile([128, 1], FP, tag="inv",
                                           name=f"iv{h}_{qb}")
                    nc.vector.reciprocal(invsum[:], ctx_ps[:, DH:DH + 1])

                    # probs = e' * invsum  -> DMA out
                    probs_sb = sb_probs.tile([128, S], BF, tag="probs",
                                             name=f"pb{h}_{qb}")
                    nc.vector.tensor_scalar_mul(probs_sb[:], e_sb[:], invsum[:])
                    nc.sync.dma_start(
                        out=probs_d.ap()[h, 128 * qb:128 * qb + 128, :],
                        in_=probs_sb[:])

                    # ctx scaled + transposed into persistent ctx^T
                    ctx_sb = sb_small.tile([128, DH], BF, tag="ctx",
                                           name=f"cs{h}_{qb}")
                    nc.vector.tensor_scalar_mul(ctx_sb[:], ctx_ps[:, 0:DH],
                                                invsum[:])
                    ctxT_ps = ps_small.tile([DH, 128], BF, tag="small",
                                            name=f"ct{h}_{qb}")
                    nc.tensor.transpose(ctxT_ps[:], ctx_sb[:], identb_sb[:])
                    nc.vector.tensor_copy(
                        ctxT_sb[64 * h2:64 * h2 + 64, hp,
                                128 * qb:128 * qb + 128],
                        ctxT_ps[:])

                def emit_wo(qb):
                    # partial out = ctx^T(qb)^T @ Wo^T  for token block qb,
                    # written to quarter buffer qb%4 at group-rank-major rows
                    psb = sb_probs.tile([128, H], BF, tag="partial",
                                        name=f"po{qb}")
                    for nk in range(2):
                        ps = ps_small.tile([128, 512], FP, tag="small",
                                           name=f"wo{qb}{nk}")
                        for cc in range(2):
                            nc.tensor.matmul(
                                ps[:],
                                lhsT=ctxT_sb[:, cc, 128 * qb:128 * qb + 128],
                                rhs=wo_sb[:, cc, 512 * nk:512 * nk + 512],
                                start=(cc == 0), stop=(cc == 1),
                            )
                        nc.scalar.copy(psb[:, 512 * nk:512 * nk + 512], ps[:])
                    g, i = divmod(qb, 4)
                    nc.sync.dma_start(
                        out=partials[i][128 * g:128 * g + 128, :], in_=psb[:])

                rs_sbs = {}

                def emit_rs(i):
                    # ReduceScatter quarter i + readback, all on gpsimd so no
                    # compute-engine FIFO ever waits on the collective chain
                    if single_core:
                        nc.gpsimd.dma_start(out=rss[i][:, :],
                                            in_=partials[i][0:128, :])
                    else:
                        nc.gpsimd.collective_compute(
                            "ReduceScatter",
                            ALU.add,
                            replica_groups=[[0, 1, 2, 3], [4, 5, 6, 7]],
                            ins=[partials[i].opt()],
                            outs=[rss[i].opt()],
                        )
                    rs_sb = sb_probs.tile([128, H], BF, tag="partial",
                                          name=f"rsb{i}")
                    nc.gpsimd.dma_start(out=rs_sb[:], in_=rss[i][:, :])
                    rs_sbs[i] = rs_sb

                def emit_ln(i):
                    rs_sb = rs_sbs[i]
                    res = sb_e.tile([128, H], FP, tag="res", name=f"res{i}")
                    nc.vector.tensor_tensor(
                        res[:], rs_sb[:], xres_sb[:, i, :], ALU.add)
                    stats = sb_small.tile([128, 2, 6], FP, tag="st",
                                          name=f"st{i}")
                    mv = sb_small.tile([128, 2], FP, tag="mv", name=f"mv{i}")
                    for a in range(2):
                        nc.vector.bn_stats(
                            stats[:, a, :], res[:, 512 * a:512 * a + 512])
                    nc.vector.bn_aggr(mv[:], stats[:])
                    lnv = sb_small.tile([128, 1], FP, tag="inv", name=f"lv{i}")
                    nc.scalar.activation(lnv[:], mv[:, 1:2], AF.Ln, bias=eps_sb[:])
                    rstd = sb_small.tile([128, 1], FP, tag="inv", name=f"rs{i}")
                    nc.scalar.activation(rstd[:], lnv[:], AF.Exp, scale=-0.5)
                    nrm = sb_e.tile([128, H], FP, tag="nrm", name=f"nm{i}")
                    nc.vector.tensor_scalar(
                        nrm[:], res[:], mv[:, 0:1], rstd[:],
                        ALU.subtract, ALU.mult)
                    nc.vector.tensor_tensor(nrm[:], nrm[:], gamma_sb[:], ALU.mult)
                    nc.vector.tensor_tensor(nrm[:], nrm[:], beta_sb[:], ALU.add)
                    nc.sync.dma_start(
                        out=norm_d.ap()[128 * i:128 * i + 128, :], in_=nrm[:])

                # software pipeline: while ACT runs exp(b), PE runs the
                # transposes/ctx of block b-1 instead of stalling. qb order is
                # residue-major so each quarter's ReduceScatter launches early.
                qb_order = sorted(range(NQB), key=lambda q: (q % 4, q // 4))
                # pair-0 heads of the first 3 q-blocks run first so attention
                # starts while pair-1 q/k (emitted mid-pipeline) compute
                blocks = [(h, qb) for qb in qb_order for h in range(HPG)]
                done_in_residue = [0, 0, 0, 0]
                rest_done = {qb: 0 for qb in range(NQB)}

                def after_wo(qb):
                    i = qb % 4
                    done_in_residue[i] += 1
                    if done_in_residue[i] == 4:
                        emit_rs(i)
                        if i >= 1:
                            emit_ln(i - 1)

                def finish(entry):
                    emit_rest(*entry)
                    qb = entry[1]
                    rest_done[qb] += 1
                    if rest_done[qb] == HPG:
                        emit_wo(qb)
                        after_wo(qb)

                p1 = p2 = None     # p1: needs transposes; p2: needs rest
                for (h, qb) in blocks:
                    e_sb = emit_scores(h, qb)
                    if p1 is not None:
                        eT = emit_transposes(*p1)
                        if p2 is not None:
                            finish(p2)
                        p2 = (*p1, eT)
                    p1 = (h, qb, e_sb)
                eT = emit_transposes(*p1)
                finish(p2)
                finish((*p1, eT))
                emit_ln(3)

    nc.compile()
    return nc


def _host_inputs(hidden_states, attention_mask, attention_weights,
                 Wq, bq, Wk, bk, Wv, bv, Wo, bo, gamma, beta):
    import ml_dtypes
    f32 = np.float32
    bf16 = ml_dtypes.bfloat16
    hs = np.asarray(hidden_states, f32)
    mask = np.asarray(attention_mask)
    aw = np.asarray(attention_weights, f32)
    Wq, bq = np.asarray(Wq, f32), np.asarray(bq, f32)
    Wk, bk = np.asarray(Wk, f32), np.asarray(bk, f32)
    Wv, bv = np.asarray(Wv, f32), np.asarray(bv, f32)
    Wo, bo = np.asarray(Wo, f32), np.asarray(bo, f32)
    gamma, beta = np.asarray(gamma, f32), np.asarray(beta, f32)

    ident = np.eye(128, dtype=f32)
    scale = f32(1.0 / np.sqrt(DH))

    def chunk_pf(a, parts=128):
        # [C*parts, F...] -> [parts, C, F...] contiguous
        C = a.shape[0] // parts
        return np.ascontiguousarray(
            a.reshape(C, parts, *a.shape[1:]).transpose(1, 0, *range(2, a.ndim + 1)))

    in_maps = []
    for core in range(8):
        b, g = divmod(core, G)
        aw_b = np.maximum(aw[b], 1e-20)
        lnaw = np.log(aw_b) + (1.0 - mask[b].astype(f32)) * f32(-10000.0)
        sl = slice(DG * g, DG * g + DG)
        lnaw_hi = lnaw.astype(bf16)
        lnaw_lo = (lnaw - lnaw_hi.astype(f32)).astype(bf16)
        in_maps.append({
            "xt": chunk_pf(np.ascontiguousarray(hs[b].T)).astype(bf16),
            "xres": chunk_pf(hs[b, TB * g:TB * g + TB] + bo[None, :]),
            "wq": chunk_pf(np.ascontiguousarray(Wq[sl].T) * scale).astype(bf16),
            "wk": chunk_pf(np.ascontiguousarray(Wk[sl].T)).astype(bf16),
            "wv": chunk_pf(np.ascontiguousarray(Wv[sl].T)).astype(bf16),
            "wo": chunk_pf(np.ascontiguousarray(Wo[:, sl].T)).astype(bf16),
            "bq": (bq[sl] * scale).reshape(1, DG).astype(bf16),
            "bk": bk[sl].reshape(1, DG).astype(bf16),
            "bv": bv[sl].reshape(1, DG).astype(bf16),
            "lnaw": np.stack([lnaw_hi, lnaw_lo]).reshape(2, S),
            "invaw": np.ascontiguousarray((1.0 / aw_b).reshape(NSC, 128).T),
            "ident": ident,
            "gamma": np.ascontiguousarray(np.broadcast_to(gamma[None, :], (128, H))),
            "beta": np.ascontiguousarray(np.broadcast_to(beta[None, :], (128, H))),
        })
    return in_maps


def kernel(**inputs):
    from concourse.bass_utils import run_bass_kernel_spmd

    if "nc" not in _CACHE:
        _CACHE["nc"] = _build_nc()
    nc = _CACHE["nc"]

    in_maps = _host_inputs(**inputs)
    res = run_bass_kernel_spmd(nc, in_maps, core_ids=list(range(8)))
    outs = res.results

    normed = np.empty((B, S, H), np.float32)
    probs = np.empty((B, NH, S, S), np.float32)
    for core in range(8):
        b, g = divmod(core, G)
        normed[b, TB * g:TB * g + TB, :] = outs[core]["norm"]
        probs[b, HPG * g:HPG * g + HPG] = outs[core]["probs"].astype(np.float32)
    return normed, probs
